# revision 20
# baseline (speedup 1.0000x reference)
"""Trainium2 Bass kernel for nn_DDINOLoss (DINO-style distillation loss).

Strategy
--------
Data-parallel over the batch dim (32 batch elems -> 4 per core on 8 cores).
Each core computes a partial scalar loss over its 4 batch elements; the host
sums the 8 partials.

Math (per (i, j) crop pair, teacher chunk i, student crop j != i):
  sum_d -t_d * log_softmax(x)_d = lse(x) - t . x      (since sum_d t_d == 1)
so the cls term needs only lse(v) and q.v, and the region term needs
lse(x_row) and t_sel . x_row where t_sel is the teacher softmax row picked by
the feature-similarity argmax.  The argmax gather is replaced by a
mask-select:  D[s, n] = x_s . E_n  (E = unnormalized teacher exp rows),
dsel[s] = sum_n (sim[s, n] == max_n sim[s, :]) * D[s, n] / Z[n].
Everything becomes matmuls + elementwise ops - no data-dependent memory ops.

v3: fp8 DoubleRow pipeline (all numerics validated on host, ~6e-5 rel err):
 * cross-entropy is shift-invariant in x, so both student and teacher logits
   are sent PRE-SHIFTED by their per-row max (host); the shifts cancel in
   lse - t.x and in D/Z exactly -> no correction constants.
 * shifted x fits fp8e4m3, exp(shifted) fits fp8e5m2: the big D matmul and
   the lse ones-matmuls run in fp8 DoubleRow (2 d-tiles contracted/pass).
 * teacher E = exp(u) emitted by ScalarE directly as fp8e5m2.
 * part of the student exps move off ScalarE to VectorE via a uint8 bit
   trick: u8 = round(40*log2e*x') + 60 bitcast as fp8e5m2 IS 2^(t) with a
   2-bit-mantissa PWL - validated, final rel err 2.4e-5.
 * crop-aware D (crop0 rows need teacher chunk 1 only, crop1 chunk 0 only),
   Z folded into D via embedded all-ones x columns.
 * mask-select fused: scalar_tensor_tensor((sim == max) * Dn, accum_out).
 * single-shot final combine (one Ln) to avoid Exp/Ln ACT-table thrash.
"""

import sys

import numpy as np

if "/opt/trn_rl_repo" not in sys.path:
    sys.path.insert(0, "/opt/trn_rl_repo")

import ml_dtypes

import concourse.bass as bass
import concourse.tile as tile
from concourse import bacc, mybir
from concourse.bass_utils import run_bass_kernel_spmd

BF16 = ml_dtypes.bfloat16
F16 = np.float16
E4M3 = ml_dtypes.float8_e4m3
F32 = np.float32

# ---- problem constants (hardcoded per spec) ----
OUT_DIM = 4096
NCROPS = 10
STUDENT_TEMP = 0.1
WARMUP_TEACHER_TEMP = 0.04
TEACHER_TEMP = 0.07
WARMUP_EPOCHS = 30
NEPOCHS = 100
B = 32
NG = 196
NL = 36
DFEAT = 384
N_CORES = 8
NB = B // N_CORES              # batch elems per core = 4
SPLIT = [NG, NG] + [NL] * (NCROPS - 2)
OFFS = np.cumsum([0] + [s * B for s in SPLIT])
TGB = 2 * NG                       # teacher region rows per batch elem = 392
TGBP = 400                         # E tile padded (16B-aligned d-step)
TG = NB * TGB                      # per-core teacher rows = 1568
DT = OUT_DIM // 128                # 32 d-tiles
FT = DFEAT // 128                  # 3 feature tiles
CHUNK_T = 4                        # d-tiles per teacher exp chunk
CHUNK_X = 8                        # d-tiles per student exp chunk
# which student-exp chunks go to the DVE bit-trick instead of ScalarE
DVE_CHUNKS = (3,)

# per-batch-elem student column layout (682 = 680 rows + 2 ones cols),
# padded to 688 for 16B-aligned fp8 d-steps.
SEGS = [("ones", 0, 1), ("c1", 0, 127), ("ones", 0, 1), ("c1", 127, 69),
        ("loc", 0, 58), ("loc", 58, 128), ("loc", 186, 102), ("c0", 0, 26),
        ("c0", 26, 128), ("c0", 154, 42), ("pad", 0, 6)]
SGB = 682                          # real student cols per batch elem
SGBP = 688                         # padded stride
SG = NB * SGBP                     # per-core student cols (padded)
# s-tiles: (col0, ms, chunk)  chunk: 0 -> teacher cols 0:196, 1 -> 196:392,
# 2 -> both.  Z0 lives at (tile0, row 0), Z1 at (tile1, row 0).
TILES = [(0, 128, 0), (128, 128, 2), (256, 128, 2), (384, 128, 2),
         (512, 128, 1), (640, 42, 1)]
NST = len(TILES)
NCOL = NB * NST                    # 24 dsel columns per core
NSUM = NB * SGB + NCROPS * NB      # 2768 log-sum-exp slots
HALVES = [(0, 384), (384, 298)]    # lse column halves

_PROG_CACHE = {}


def _temp_from_epoch(epoch):
    sched = np.concatenate(
        (np.linspace(WARMUP_TEACHER_TEMP, TEACHER_TEMP, WARMUP_EPOCHS),
         np.ones(NEPOCHS - WARMUP_EPOCHS) * TEACHER_TEMP))
    return float(sched[int(epoch)])


def _chunk_cols(chunk):
    """(n0, w) teacher-col range streamed for a tile chunk code."""
    return (0, 196) if chunk == 0 else ((196, 196) if chunk == 1 else (0, 392))


# ---------------------------------------------------------------------------
# device program
# ---------------------------------------------------------------------------

def _build_program(st):
    """st = 1/teacher_temp. Returns (nc, out_name)."""
    fp32 = mybir.dt.float32
    fp16 = mybir.dt.float16
    bf16 = mybir.dt.bfloat16
    f8e4 = mybir.dt.float8e4
    f8e5 = mybir.dt.float8e5
    u8 = mybir.dt.uint8
    Exp = mybir.ActivationFunctionType.Exp
    Ln = mybir.ActivationFunctionType.Ln
    AX = mybir.AxisListType.X
    OP = mybir.AluOpType
    DR = mybir.MatmulPerfMode.DoubleRow

    nc = bacc.Bacc("TRN2", debug=False)

    xt_d = nc.dram_tensor("xt", [OUT_DIM, SG], f8e4, kind="ExternalInput")
    trt_d = nc.dram_tensor("trt", [OUT_DIM, TG], fp16, kind="ExternalInput")
    sft_d = nc.dram_tensor("sft", [DFEAT, SG], bf16, kind="ExternalInput")
    tftn_d = nc.dram_tensor("tftn", [DFEAT, TG], bf16, kind="ExternalInput")
    sct_d = nc.dram_tensor("sctt", [128, DT * 41], bf16, kind="ExternalInput")
    tct_d = nc.dram_tensor("tctt", [128, DT * 8], bf16, kind="ExternalInput")
    wall_d = nc.dram_tensor("wall", [1, NSUM], fp32, kind="ExternalInput")
    w0p_d = nc.dram_tensor("w0p", [128, NCOL], fp32, kind="ExternalInput")
    w1p_d = nc.dram_tensor("w1p", [128, NCOL], fp32, kind="ExternalInput")
    wq_d = nc.dram_tensor("wq", [8, NCROPS * NB], fp32, kind="ExternalInput")
    out_d = nc.dram_tensor("out", [1, 1], fp32, kind="ExternalOutput")

    with tile.TileContext(nc) as tc:
        with (
            tc.tile_pool(name="xtp", bufs=2) as xtp,
            tc.tile_pool(name="trtp", bufs=2) as trtp,
            tc.tile_pool(name="ep", bufs=2) as ep,
            tc.tile_pool(name="expxp", bufs=2) as expxp,
            tc.tile_pool(name="sftp", bufs=2) as sftp,
            tc.tile_pool(name="tftp", bufs=2) as tftp,
            tc.tile_pool(name="smalls", bufs=1) as smalls,
            tc.tile_pool(name="work", bufs=2) as work,
            tc.tile_pool(name="dps", bufs=3, space="PSUM") as dpsp,
            tc.tile_pool(name="sps", bufs=2, space="PSUM") as spsp,
            tc.tile_pool(name="lseps", bufs=2, space="PSUM") as lsepsp,
            tc.tile_pool(name="miscps", bufs=1, space="PSUM") as miscpsp,
        ):
            # ---- constants / small inputs ----
            ones_b = smalls.tile([128, 1], bf16)
            nc.vector.memset(ones_b, 1.0)
            ones_f = smalls.tile([128, 1], fp32)
            nc.vector.memset(ones_f, 1.0)
            ones_row = smalls.tile([1, 128], bf16)
            nc.vector.memset(ones_row, 1.0)
            # lse DoubleRow weights: Ko step must be 16B-aligned, so allocate
            # [128, 2, 16] and slice the first column.
            ones_82f = smalls.tile([128, 2, 16], f8e5)
            nc.vector.memset(ones_82f, 1.0)
            ones_82 = ones_82f[:, :, 0:1]

            w0p_sb = smalls.tile([128, NCOL], fp32)
            nc.sync.dma_start(out=w0p_sb, in_=w0p_d.ap())
            w1p_sb = smalls.tile([128, NCOL], fp32)
            nc.sync.dma_start(out=w1p_sb, in_=w1p_d.ap())
            wq_sb = smalls.tile([8, NCROPS * NB], fp32)
            nc.sync.dma_start(out=wq_sb, in_=wq_d.ap())
            wall_sb = smalls.tile([1, NSUM], fp32)
            nc.sync.dma_start(out=wall_sb, in_=wall_d.ap())

            ds0 = smalls.tile([128, NCOL], fp32)
            nc.vector.memset(ds0, 0.0)
            ds1 = smalls.tile([128, NCOL], fp32)
            nc.vector.memset(ds1, 0.0)
            # sums-of-exps collected here; Ln + weight-reduce at the end
            sumexp = smalls.tile([1, NSUM], fp32)

            # ---- cls part ----
            sct_sb = smalls.tile([128, DT * 41], bf16)
            nc.sync.dma_start(out=sct_sb, in_=sct_d.ap())
            tct_sb = smalls.tile([128, DT * 8], bf16)
            nc.sync.dma_start(out=tct_sb, in_=tct_d.ap())

            qun = smalls.tile([128, DT * 8], bf16)
            nc.scalar.activation(qun, tct_sb, Exp, scale=st)
            expv = smalls.tile([128, DT * 41], bf16)
            nc.scalar.activation(expv, sct_sb, Exp, scale=1.0 / STUDENT_TEMP)

            # dotq[i, :40] = q_un_i . sc_j ; col 40 = Zq_i  (ones col in sctt)
            dotq_ps = miscpsp.tile([8, 41], fp32, tag="misc")
            for t in range(DT):
                nc.tensor.matmul(dotq_ps, qun[:, t * 8:(t + 1) * 8],
                                 sct_sb[:, t * 41:(t + 1) * 41],
                                 start=(t == 0), stop=(t == DT - 1))
            invzq = smalls.tile([8, 1], fp32)
            nc.vector.reciprocal(invzq, dotq_ps[:, 40:41])
            dotn = smalls.tile([8, NCROPS * NB], fp32)
            nc.vector.tensor_scalar(dotn, dotq_ps[:, 0:NCROPS * NB], invzq,
                                    None, op0=OP.mult)
            junkq = smalls.tile([8, NCROPS * NB], fp32)
            clsneg = smalls.tile([8, 1], fp32)
            nc.vector.tensor_tensor(junkq, dotn, wq_sb, op=OP.mult)
            nc.vector.tensor_reduce(clsneg, junkq, axis=AX, op=OP.add)

            # sum_d exp(10*sc): ones-matmul then fold the 32 d-tiles
            NV = DT * 41  # 1312
            sv_sb = smalls.tile([1, NV], fp32)
            for n0 in range(0, NV, 512):
                n1 = min(n0 + 512, NV)
                sv_ps = miscpsp.tile([1, 512], fp32, tag="misc")
                nc.tensor.matmul(sv_ps[:, :n1 - n0], ones_b, expv[:, n0:n1],
                                 start=True, stop=True)
                nc.vector.tensor_copy(sv_sb[:, n0:n1], sv_ps[:, :n1 - n0])
            svv = sv_sb[:, :].rearrange("p (t j) -> p t j", t=DT)
            nc.vector.tensor_reduce(
                sumexp[:, NB * SGB:NSUM],
                svv[:, :, 0:NCROPS * NB].rearrange("p t j -> p j t"),
                axis=AX, op=OP.add)

            # ---- region part, per batch element ----
            LNSHIFT = 64
            C1 = float(4.0 * (1.0 / STUDENT_TEMP) * np.log2(np.e))
            for bb in range(NB):
                # teacher E = exp(u) -> fp8e5m2, d-step padded to 400.
                # DMA issue order: first 2 teacher chunks, then the student
                # tensors, then the rest -- keeps both E and xtb early so the
                # D stream (PE critical path) starts ASAP.
                E = ep.tile([128, DT, TGBP], f8e5, tag="E")

                def _teacher_chunk(c):
                    trtc = trtp.tile([128, CHUNK_T, TGB], fp16, tag="trtc")
                    nc.sync.dma_start(
                        out=trtc,
                        in_=trt_d.ap()[c * CHUNK_T * 128:(c + 1) * CHUNK_T * 128,
                                       bb * TGB:(bb + 1) * TGB]
                        .rearrange("(t p) n -> p t n", p=128))
                    nc.scalar.activation(
                        E[:, c * CHUNK_T:(c + 1) * CHUNK_T, 0:TGB],
                        trtc, Exp, scale=1.0)

                _teacher_chunk(0)
                _teacher_chunk(1)
                xtb = xtp.tile([128, DT, SGBP], f8e4, tag="xtb")
                nc.sync.dma_start(
                    out=xtb,
                    in_=xt_d.ap()[:, bb * SGBP:(bb + 1) * SGBP]
                    .rearrange("(t p) s -> p t s", p=128))
                sftb = sftp.tile([128, FT, SGBP], bf16, tag="sftb")
                nc.sync.dma_start(
                    out=sftb,
                    in_=sft_d.ap()[:, bb * SGBP:(bb + 1) * SGBP]
                    .rearrange("(t p) s -> p t s", p=128))
                for c in range(2, DT // CHUNK_T):
                    _teacher_chunk(c)
                tftb = tftp.tile([128, FT, TGB], bf16, tag="tftb")
                nc.sync.dma_start(
                    out=tftb,
                    in_=tftn_d.ap()[:, bb * TGB:(bb + 1) * TGB]
                    .rearrange("(t p) n -> p t n", p=128))

                # DVE bit-trick expx chunks made up-front: keeps the DVE's
                # copy of the work ahead of its select backlog so the lse
                # matmuls never stall on it
                expx_pre = {}
                for c in DVE_CHUNKS:
                    expx = expxp.tile([128, CHUNK_X, SGBP], f8e5, tag="expx")
                    nc.vector.tensor_scalar(
                        expx.bitcast(u8)[:, :, 0:SGB],
                        xtb[:, c * CHUNK_X:(c + 1) * CHUNK_X, 0:SGB],
                        C1, 60.0, op0=OP.mult, op1=OP.add)
                    expx_pre[c] = expx

                zrow = work.tile([1, TGB], bf16, tag="zrow")
                invzbc = work.tile([128, TGB], fp32, tag="invzbc")
                dtiles = {}

                # D + sim matmuls per s-tile (fp8 operands; Z folded via the
                # embedded ones columns).  DoubleRow only fits tiles with
                # <=64 output rows (128-weight-element ISA cap) -> t5 only.
                for ti, (c0, ms, chunk) in enumerate(TILES):
                    n0, w = _chunk_cols(chunk)
                    dps = dpsp.tile([128, TGB], fp32, tag="dps")
                    for d in range(DT):
                        nc.tensor.matmul(dps[:ms, 0:w],
                                         xtb[:, d, c0:c0 + ms],
                                         E[:, d, n0:n0 + w],
                                         start=(d == 0),
                                         stop=(d == DT - 1))
                    sps = spsp.tile([128, TGB], fp32, tag="sps")
                    for f in range(FT):
                        nc.tensor.matmul(sps[:ms, 0:w],
                                         sftb[:, f, c0:c0 + ms],
                                         tftb[:, f, n0:n0 + w],
                                         start=(f == 0), stop=(f == FT - 1))
                    dtiles[ti] = (dps, sps, ms, chunk)

                    if ti == 0:
                        nc.vector.tensor_copy(zrow[:, 0:196], dps[0:1, 0:196])
                    if ti == 1:
                        nc.vector.tensor_copy(zrow[:, 196:392],
                                              dps[0:1, 196:392])
                        bc_ps = miscpsp.tile([128, TGB], fp32, tag="misc")
                        nc.tensor.matmul(bc_ps, ones_row, zrow,
                                         start=True, stop=True)
                        nc.vector.reciprocal(invzbc, bc_ps)

                    # drain completed tiles once invzbc is available (ti>=1)
                    if ti >= 1:
                        for tj in sorted(dtiles):
                            dpsj, spsj, msj, chj = dtiles.pop(tj)
                            col = bb * NST + tj
                            n0j, wj = _chunk_cols(chj)
                            Dn = work.tile([128, TGB], fp32, tag="Dn")
                            nc.vector.tensor_tensor(
                                Dn[:msj, 0:wj], dpsj[:msj, 0:wj],
                                invzbc[:msj, n0j:n0j + wj], op=OP.mult)
                            if chj in (0, 2):
                                m0 = work.tile([128, 1], fp32, tag="m0")
                                nc.vector.tensor_reduce(
                                    m0[:msj], spsj[:msj, 0:196],
                                    axis=AX, op=OP.max)
                                junk = work.tile([128, 196], fp32, tag="junk")
                                nc.vector.scalar_tensor_tensor(
                                    junk[:msj], spsj[:msj, 0:196], m0[:msj],
                                    Dn[:msj, 0:196], op0=OP.is_equal,
                                    op1=OP.mult,
                                    accum_out=ds0[:msj, col:col + 1])
                            if chj in (1, 2):
                                so = 196 if chj == 2 else 0
                                m1 = work.tile([128, 1], fp32, tag="m1")
                                nc.vector.tensor_reduce(
                                    m1[:msj], spsj[:msj, so:so + 196],
                                    axis=AX, op=OP.max)
                                junk1 = work.tile([128, 196], fp32,
                                                  tag="junk1")
                                nc.vector.scalar_tensor_tensor(
                                    junk1[:msj], spsj[:msj, so:so + 196],
                                    m1[:msj], Dn[:msj, so:so + 196],
                                    op0=OP.is_equal, op1=OP.mult,
                                    accum_out=ds1[:msj, col:col + 1])

                # student exps (fp8e5 out) + lse DoubleRow ones-matmuls;
                # after the D/sim stream so the PE isn't gated on expx early
                lse0 = lsepsp.tile([1, 512], fp32, tag="lse")
                lse1 = lsepsp.tile([1, 512], fp32, tag="lse")
                for c in range(DT // CHUNK_X):
                    expx = expxp.tile([128, CHUNK_X, SGBP], f8e5, tag="expx")
                    if c in DVE_CHUNKS:
                        nc.vector.tensor_scalar(
                            expx.bitcast(u8)[:, :, 0:SGB],
                            xtb[:, c * CHUNK_X:(c + 1) * CHUNK_X, 0:SGB],
                            C1, 60.0, op0=OP.mult, op1=OP.add)
                    else:
                        nc.scalar.activation(
                            expx[:, :, 0:SGB],
                            xtb[:, c * CHUNK_X:(c + 1) * CHUNK_X, 0:SGB],
                            Exp, scale=1.0 / STUDENT_TEMP)
                    for jj in range(CHUNK_X // 2):
                        p = c * (CHUNK_X // 2) + jj
                        first, last = (p == 0), (p == DT // 2 - 1)
                        nc.tensor.matmul(
                            lse0[:, 0:HALVES[0][1]], ones_82,
                            expx[:, 2 * jj:2 * jj + 2, 0:384],
                            start=first, stop=last, perf_mode=DR)
                        nc.tensor.matmul(
                            lse1[:, 0:HALVES[1][1]], ones_82,
                            expx[:, 2 * jj:2 * jj + 2, 384:682],
                            start=first, stop=last, perf_mode=DR)
                nc.vector.tensor_copy(
                    sumexp[:, bb * SGB:bb * SGB + 384], lse0[:, 0:384])
                nc.vector.tensor_copy(
                    sumexp[:, bb * SGB + 384:(bb + 1) * SGB], lse1[:, 0:298])

                if bb == NB - 2:
                    # early positive-combine over b0..b2 slots: overlaps the
                    # last batch elem's compute (costs one ACT table swap)
                    NA = (NB - 1) * SGB
                    logsA = smalls.tile([1, NA], fp32)
                    nc.scalar.activation(logsA, sumexp[:, 0:NA], Ln,
                                         scale=2.0 ** -LNSHIFT)
                    junkA = smalls.tile([1, NA], fp32)
                    accA = smalls.tile([1, 1], fp32)
                    nc.vector.scalar_tensor_tensor(
                        junkA, logsA, 1.0, wall_sb[:, 0:NA],
                        op0=OP.mult, op1=OP.mult, accum_out=accA)

            # ---- final combine for the remaining slots
            NA = (NB - 1) * SGB
            logs = smalls.tile([1, NSUM - NA], fp32)
            nc.scalar.activation(logs, sumexp[:, NA:NSUM], Ln,
                                 scale=2.0 ** -LNSHIFT)
            junk_p = smalls.tile([1, NSUM - NA], fp32)
            accB = smalls.tile([1, 1], fp32)
            nc.vector.scalar_tensor_tensor(junk_p, logs, 1.0,
                                           wall_sb[:, NA:NSUM],
                                           op0=OP.mult, op1=OP.mult,
                                           accum_out=accB)
            acc = smalls.tile([1, 1], fp32)
            nc.vector.tensor_tensor(acc, accA, accB, op=OP.add)

            # negative part: dsel columns dotted with packed weights + cls
            junkr0 = smalls.tile([128, NCOL], fp32)
            ra0 = smalls.tile([128, 1], fp32)
            nc.vector.scalar_tensor_tensor(junkr0, ds0, 1.0, w0p_sb,
                                           op0=OP.mult, op1=OP.mult,
                                           accum_out=ra0)
            junkr1 = smalls.tile([128, NCOL], fp32)
            ra1 = smalls.tile([128, 1], fp32)
            nc.vector.scalar_tensor_tensor(junkr1, ds1, 1.0, w1p_sb,
                                           op0=OP.mult, op1=OP.mult,
                                           accum_out=ra1)
            ra = smalls.tile([128, 1], fp32)
            nc.vector.tensor_tensor(ra, ra0, ra1, op=OP.add)
            ra2 = smalls.tile([128, 1], fp32)
            nc.vector.tensor_copy(ra2, ra)
            nc.vector.tensor_tensor(ra2[0:8], ra[0:8], clsneg, op=OP.add)

            neg_ps = miscpsp.tile([1, 1], fp32, tag="misc")
            nc.tensor.matmul(neg_ps, ones_f, ra2, start=True, stop=True)
            wall_np, _, _, _ = _make_weights()
            lncomp = float(LNSHIFT * np.log(2.0) * wall_np.sum())
            accc = smalls.tile([1, 1], fp32)
            nc.vector.tensor_scalar(accc, acc, lncomp, None, op0=OP.add)
            res = smalls.tile([1, 1], fp32)
            nc.vector.tensor_tensor(res, accc, neg_ps, op=OP.subtract)
            nc.sync.dma_start(out=out_d.ap(), in_=res)

    nc.compile()
    return nc, "out"


# ---------------------------------------------------------------------------
# host-side sharding / input prep
# ---------------------------------------------------------------------------

def _col_kinds():
    kinds = []
    for kind, _, ln in SEGS:
        kinds += [kind] * ln
    return kinds


def _make_weights():
    c = 1.0 / (18.0 * 32.0)
    kinds = _col_kinds()[:SGB]
    Wl = np.zeros(SGB, F32)
    W0 = np.zeros(SGB, F32)
    W1 = np.zeros(SGB, F32)
    for k, kind in enumerate(kinds):
        if kind == "c0":
            Wl[k] = 1 * 0.5 * c / NG
            W1[k] = 10.0 * 0.5 * c / NG
        elif kind == "c1":
            Wl[k] = 1 * 0.5 * c / NG
            W0[k] = 10.0 * 0.5 * c / NG
        elif kind == "loc":
            Wl[k] = 2 * 0.5 * c / NL
            W0[k] = 10.0 * 0.5 * c / NL
            W1[k] = 10.0 * 0.5 * c / NL
    wv = np.array([(2 if j >= 2 else 1) * 0.5 * c for j in range(NCROPS)], F32)
    wq = np.zeros((2 * NB, NCROPS * NB), F32)
    for i in range(2):
        for bb in range(NB):
            for j in range(NCROPS):
                if j != i:
                    wq[i * NB + bb, j * NB + bb] = 10.0 * 0.5 * c

    def col_pack(W):  # [682] -> [128, 6] zero-padded, tiled x NB
        w6 = np.zeros((128, NST), F32)
        for ti, (c0, ms, _) in enumerate(TILES):
            w6[:ms, ti] = W[c0:c0 + ms]
        return np.ascontiguousarray(np.tile(w6, (1, NB)))

    wall = np.concatenate([np.tile(Wl, NB), np.repeat(wv, NB)])[None, :]
    return np.ascontiguousarray(wall), col_pack(W0), col_pack(W1), \
        np.ascontiguousarray(wq)


def _dtile_pack(a):
    """[4096, m] -> [128, 32*m] with block t = rows [128t, 128t+128)."""
    d, m = a.shape
    t = d // 128
    return np.ascontiguousarray(
        a.reshape(t, 128, m).transpose(1, 0, 2).reshape(128, t * m))


def _student_rows(bb):
    """Global SR row index per packed column (one b); -1 ones, -2 pad."""
    loc = np.concatenate(
        [np.arange(OFFS[j] + bb * NL, OFFS[j] + (bb + 1) * NL)
         for j in range(2, NCROPS)])
    base = {"c0": np.arange(OFFS[0] + bb * NG, OFFS[0] + (bb + 1) * NG),
            "c1": np.arange(OFFS[1] + bb * NG, OFFS[1] + (bb + 1) * NG),
            "loc": loc}
    idx = []
    for kind, s0, ln in SEGS:
        if kind == "ones":
            idx.append(np.full(ln, -1, np.int64))
        elif kind == "pad":
            idx.append(np.full(ln, -2, np.int64))
        else:
            idx.append(base[kind][s0:s0 + ln])
    return np.concatenate(idx)


def _teacher_rows(bb):
    return np.concatenate([np.arange(bb * NG, (bb + 1) * NG),
                           np.arange(B * NG + bb * NG, B * NG + (bb + 1) * NG)])


def _gather_cols(A, idx, ones_fill, dtype):
    """A [rows, dim] -> [dim, len(idx)] transposed gather.
    idx -1 -> ones_fill, -2 -> 0."""
    out = np.empty((A.shape[1], len(idx)), dtype)
    valid = idx >= 0
    out[:, valid] = A[idx[valid]].T
    out[:, idx == -1] = ones_fill
    out[:, idx == -2] = 0.0
    return np.ascontiguousarray(out)


def _prepare_in_maps(student_cls_pred, student_region_pred, student_feats,
                     teacher_cls_pred, teacher_region_pred, teacher_feats,
                     center, center_grid):
    SR = np.asarray(student_region_pred, F32)
    SF = np.asarray(student_feats, F32)
    TR = np.asarray(teacher_region_pred, F32)
    TF = np.asarray(teacher_feats, F32)
    SC = np.asarray(student_cls_pred, F32)
    TC = np.asarray(teacher_cls_pred, F32)
    center = np.asarray(center, F32).reshape(-1)
    cg = np.asarray(center_grid, F32).reshape(-1)

    if np.any(cg != 0):
        TR = TR - cg[None, :]
    TC = TC - center[None, :]

    # shift-invariance of CE: pre-subtract per-row maxes (cancels exactly)
    SRs = SR - SR.max(1, keepdims=True)
    temp = None  # teacher shift+scale applied per-core below (st known there)

    tfn = TF / np.maximum(np.sqrt((TF * TF).sum(1, keepdims=True)), 1e-12)

    wall, w0p, w1p, wq = _make_weights()

    srows = [_student_rows(bb) for bb in range(B)]
    trows = [_teacher_rows(bb) for bb in range(B)]

    in_maps = []
    for core in range(N_CORES):
        bbs = range(core * NB, (core + 1) * NB)
        sr_idx = np.concatenate([srows[bb] for bb in bbs])
        tr_idx = np.concatenate([trows[bb] for bb in bbs])
        xt = _gather_cols(SRs, sr_idx, 1.0, E4M3)
        sft = _gather_cols(SF, sr_idx, 0.0, BF16)
        TRc = TR[tr_idx]
        u = (TRc - TRc.max(1, keepdims=True)) * _ST_GLOBAL[0]
        trt = np.ascontiguousarray(u.T).astype(F16)
        tftn = np.ascontiguousarray(tfn[tr_idx].T).astype(BF16)
        sc_rows = SC[[j * B + bb for j in range(NCROPS) for bb in bbs]]
        tc_rows = TC[[i * B + bb for i in range(2) for bb in bbs]]
        sc_aug = np.concatenate(
            [sc_rows.T, np.ones((OUT_DIM, 1), F32)], axis=1)  # [4096, 41]
        in_maps.append({
            "xt": xt,
            "trt": trt,
            "sft": sft,
            "tftn": tftn,
            "sctt": _dtile_pack(sc_aug).astype(BF16),
            "tctt": _dtile_pack(np.ascontiguousarray(tc_rows.T)).astype(BF16),
            "wall": wall,
            "w0p": w0p,
            "w1p": w1p,
            "wq": wq,
        })
    return in_maps


_ST_GLOBAL = [1.0 / TEACHER_TEMP]


def _get_program(st):
    key = round(st, 9)
    if key not in _PROG_CACHE:
        _PROG_CACHE[key] = _build_program(st)
    return _PROG_CACHE[key]


def run_cores(inputs, trace=False, **kw):
    """Build+run on 8 cores; returns (partials[8], BassKernelResults)."""
    temp = _temp_from_epoch(inputs["epoch"])
    _ST_GLOBAL[0] = 1.0 / temp
    nc, out_name = _get_program(1.0 / temp)
    in_maps = _prepare_in_maps(
        inputs["student_cls_pred"], inputs["student_region_pred"],
        inputs["student_feats"], inputs["teacher_cls_pred"],
        inputs["teacher_region_pred"], inputs["teacher_feats"],
        inputs["center"], inputs["center_grid"])
    res = run_bass_kernel_spmd(nc, in_maps, core_ids=list(range(N_CORES)),
                               trace=trace, **kw)
    partials = [float(r[out_name].reshape(-1)[0]) for r in res.results]
    return partials, res


def kernel(**inputs) -> np.ndarray:
    assert int(inputs["n_global"]) == NG and int(inputs["n_local"]) == NL
    partials, _ = run_cores(inputs)
    return np.float32(sum(partials))


# revision 21
# speedup vs baseline: 1.0588x; 1.0588x over previous
"""Trainium2 Bass kernel for nn_DDINOLoss (DINO-style distillation loss).

Strategy
--------
Data-parallel over the batch dim (32 batch elems -> 4 per core on 8 cores).
Each core computes a partial scalar loss over its 4 batch elements; the host
sums the 8 partials.

Math (per (i, j) crop pair, teacher chunk i, student crop j != i):
  sum_d -t_d * log_softmax(x)_d = lse(x) - t . x      (since sum_d t_d == 1)
so the cls term needs only lse(v) and q.v, and the region term needs
lse(x_row) and t_sel . x_row where t_sel is the teacher softmax row picked by
the feature-similarity argmax.  The argmax gather is replaced by a
mask-select:  D[s, n] = x_s . E_n  (E = unnormalized teacher exp rows),
dsel[s] = sum_n (sim[s, n] == max_n sim[s, :]) * D[s, n] / Z[n].
Everything becomes matmuls + elementwise ops - no data-dependent memory ops.

v3: fp8 DoubleRow pipeline (all numerics validated on host, ~6e-5 rel err):
 * cross-entropy is shift-invariant in x, so both student and teacher logits
   are sent PRE-SHIFTED by their per-row max (host); the shifts cancel in
   lse - t.x and in D/Z exactly -> no correction constants.
 * shifted x fits fp8e4m3, exp(shifted) fits fp8e5m2: the big D matmul and
   the lse ones-matmuls run in fp8 DoubleRow (2 d-tiles contracted/pass).
 * teacher E = exp(u) emitted by ScalarE directly as fp8e5m2.
 * part of the student exps move off ScalarE to VectorE via a uint8 bit
   trick: u8 = round(40*log2e*x') + 60 bitcast as fp8e5m2 IS 2^(t) with a
   2-bit-mantissa PWL - validated, final rel err 2.4e-5.
 * crop-aware D (crop0 rows need teacher chunk 1 only, crop1 chunk 0 only),
   Z folded into D via embedded all-ones x columns.
 * mask-select fused: scalar_tensor_tensor((sim == max) * Dn, accum_out).
 * single-shot final combine (one Ln) to avoid Exp/Ln ACT-table thrash.
"""

import sys

import numpy as np

if "/opt/trn_rl_repo" not in sys.path:
    sys.path.insert(0, "/opt/trn_rl_repo")

import ml_dtypes

import concourse.bass as bass
import concourse.tile as tile
from concourse import bacc, mybir
from concourse.bass_utils import run_bass_kernel_spmd

BF16 = ml_dtypes.bfloat16
F16 = np.float16
E4M3 = ml_dtypes.float8_e4m3
F32 = np.float32

# ---- problem constants (hardcoded per spec) ----
OUT_DIM = 4096
NCROPS = 10
STUDENT_TEMP = 0.1
WARMUP_TEACHER_TEMP = 0.04
TEACHER_TEMP = 0.07
WARMUP_EPOCHS = 30
NEPOCHS = 100
B = 32
NG = 196
NL = 36
DFEAT = 384
N_CORES = 8
NB = B // N_CORES              # batch elems per core = 4
SPLIT = [NG, NG] + [NL] * (NCROPS - 2)
OFFS = np.cumsum([0] + [s * B for s in SPLIT])
TGB = 2 * NG                       # teacher region rows per batch elem = 392
TGBP = 400                         # E tile padded (16B-aligned d-step)
TG = NB * TGB                      # per-core teacher rows = 1568
DT = OUT_DIM // 128                # 32 d-tiles
FT = DFEAT // 128                  # 3 feature tiles
CHUNK_T = 4                        # d-tiles per teacher exp chunk
CHUNK_X = 8                        # d-tiles per student exp chunk
# which student-exp chunks go to the DVE bit-trick instead of ScalarE
DVE_CHUNKS = (3,)

# per-batch-elem student column layout (682 = 680 rows + 2 ones cols),
# padded to 688 for 16B-aligned fp8 d-steps.
SEGS = [("ones", 0, 1), ("c1", 0, 127), ("ones", 0, 1), ("c1", 127, 69),
        ("loc", 0, 58), ("loc", 58, 128), ("loc", 186, 102), ("c0", 0, 26),
        ("c0", 26, 128), ("c0", 154, 42), ("pad", 0, 6)]
SGB = 682                          # real student cols per batch elem
SGBP = 688                         # padded stride
SG = NB * SGBP                     # per-core student cols (padded)
# s-tiles: (col0, ms, chunk)  chunk: 0 -> teacher cols 0:196, 1 -> 196:392,
# 2 -> both.  Z0 lives at (tile0, row 0), Z1 at (tile1, row 0).
TILES = [(0, 128, 0), (128, 128, 2), (256, 128, 2), (384, 128, 2),
         (512, 128, 1), (640, 42, 1)]
NST = len(TILES)
NCOL = NB * NST                    # 24 dsel columns per core
NSUM = NB * SGB + NCROPS * NB      # 2768 log-sum-exp slots
HALVES = [(0, 384), (384, 298)]    # lse column halves

_PROG_CACHE = {}


def _temp_from_epoch(epoch):
    sched = np.concatenate(
        (np.linspace(WARMUP_TEACHER_TEMP, TEACHER_TEMP, WARMUP_EPOCHS),
         np.ones(NEPOCHS - WARMUP_EPOCHS) * TEACHER_TEMP))
    return float(sched[int(epoch)])


def _chunk_cols(chunk):
    """(n0, w) teacher-col range streamed for a tile chunk code."""
    return (0, 196) if chunk == 0 else ((196, 196) if chunk == 1 else (0, 392))


# ---------------------------------------------------------------------------
# device program
# ---------------------------------------------------------------------------

def _build_program(st):
    """st = 1/teacher_temp. Returns (nc, out_name)."""
    fp32 = mybir.dt.float32
    fp16 = mybir.dt.float16
    bf16 = mybir.dt.bfloat16
    f8e4 = mybir.dt.float8e4
    f8e5 = mybir.dt.float8e5
    u8 = mybir.dt.uint8
    Exp = mybir.ActivationFunctionType.Exp
    Ln = mybir.ActivationFunctionType.Ln
    AX = mybir.AxisListType.X
    OP = mybir.AluOpType
    DR = mybir.MatmulPerfMode.DoubleRow

    nc = bacc.Bacc("TRN2", debug=False)

    xt_d = nc.dram_tensor("xt", [OUT_DIM, SG], f8e4, kind="ExternalInput")
    trt_d = nc.dram_tensor("trt", [OUT_DIM, TG], fp16, kind="ExternalInput")
    sft_d = nc.dram_tensor("sft", [DFEAT, SG], bf16, kind="ExternalInput")
    tftn_d = nc.dram_tensor("tftn", [DFEAT, TG], bf16, kind="ExternalInput")
    sct_d = nc.dram_tensor("sctt", [128, DT * 41], bf16, kind="ExternalInput")
    tct_d = nc.dram_tensor("tctt", [128, DT * 8], bf16, kind="ExternalInput")
    wall_d = nc.dram_tensor("wall", [1, NSUM], fp32, kind="ExternalInput")
    w0p_d = nc.dram_tensor("w0p", [128, NCOL], fp32, kind="ExternalInput")
    w1p_d = nc.dram_tensor("w1p", [128, NCOL], fp32, kind="ExternalInput")
    wq_d = nc.dram_tensor("wq", [8, NCROPS * NB], fp32, kind="ExternalInput")
    out_d = nc.dram_tensor("out", [1, 1], fp32, kind="ExternalOutput")

    with tile.TileContext(nc) as tc:
        with (
            tc.tile_pool(name="xtp", bufs=2) as xtp,
            tc.tile_pool(name="trtp", bufs=2) as trtp,
            tc.tile_pool(name="ep", bufs=2) as ep,
            tc.tile_pool(name="expxp", bufs=4) as expxp,
            tc.tile_pool(name="sftp", bufs=2) as sftp,
            tc.tile_pool(name="tftp", bufs=2) as tftp,
            tc.tile_pool(name="smalls", bufs=1) as smalls,
            tc.tile_pool(name="work", bufs=2) as work,
            tc.tile_pool(name="dps", bufs=3, space="PSUM") as dpsp,
            tc.tile_pool(name="sps", bufs=2, space="PSUM") as spsp,
            tc.tile_pool(name="lseps", bufs=2, space="PSUM") as lsepsp,
            tc.tile_pool(name="miscps", bufs=1, space="PSUM") as miscpsp,
        ):
            # ---- constants / small inputs ----
            ones_b = smalls.tile([128, 1], bf16)
            nc.vector.memset(ones_b, 1.0)
            ones_f = smalls.tile([128, 1], fp32)
            nc.vector.memset(ones_f, 1.0)
            ones_row = smalls.tile([1, 128], bf16)
            nc.vector.memset(ones_row, 1.0)
            # lse DoubleRow weights: Ko step must be 16B-aligned, so allocate
            # [128, 2, 16] and slice the first column.
            ones_82f = smalls.tile([128, 2, 16], f8e5)
            nc.vector.memset(ones_82f, 1.0)
            ones_82 = ones_82f[:, :, 0:1]

            w0p_sb = smalls.tile([128, NCOL], fp32)
            nc.sync.dma_start(out=w0p_sb, in_=w0p_d.ap())
            w1p_sb = smalls.tile([128, NCOL], fp32)
            nc.sync.dma_start(out=w1p_sb, in_=w1p_d.ap())
            wq_sb = smalls.tile([8, NCROPS * NB], fp32)
            nc.sync.dma_start(out=wq_sb, in_=wq_d.ap())
            wall_sb = smalls.tile([1, NSUM], fp32)
            nc.sync.dma_start(out=wall_sb, in_=wall_d.ap())

            ds0 = smalls.tile([128, NCOL], fp32)
            nc.vector.memset(ds0, 0.0)
            ds1 = smalls.tile([128, NCOL], fp32)
            nc.vector.memset(ds1, 0.0)
            # sums-of-exps collected here; Ln + weight-reduce at the end
            sumexp = smalls.tile([1, NSUM], fp32)

            # ---- cls part ----
            sct_sb = smalls.tile([128, DT * 41], bf16)
            nc.sync.dma_start(out=sct_sb, in_=sct_d.ap())
            tct_sb = smalls.tile([128, DT * 8], bf16)
            nc.sync.dma_start(out=tct_sb, in_=tct_d.ap())

            qun = smalls.tile([128, DT * 8], bf16)
            nc.scalar.activation(qun, tct_sb, Exp, scale=st)
            expv = smalls.tile([128, DT * 41], bf16)
            nc.scalar.activation(expv, sct_sb, Exp, scale=1.0 / STUDENT_TEMP)

            # dotq[i, :40] = q_un_i . sc_j ; col 40 = Zq_i  (ones col in sctt)
            dotq_ps = miscpsp.tile([8, 41], fp32, tag="misc")
            for t in range(DT):
                nc.tensor.matmul(dotq_ps, qun[:, t * 8:(t + 1) * 8],
                                 sct_sb[:, t * 41:(t + 1) * 41],
                                 start=(t == 0), stop=(t == DT - 1))
            invzq = smalls.tile([8, 1], fp32)
            nc.vector.reciprocal(invzq, dotq_ps[:, 40:41])
            dotn = smalls.tile([8, NCROPS * NB], fp32)
            nc.vector.tensor_scalar(dotn, dotq_ps[:, 0:NCROPS * NB], invzq,
                                    None, op0=OP.mult)
            junkq = smalls.tile([8, NCROPS * NB], fp32)
            clsneg = smalls.tile([8, 1], fp32)
            nc.vector.tensor_tensor(junkq, dotn, wq_sb, op=OP.mult)
            nc.vector.tensor_reduce(clsneg, junkq, axis=AX, op=OP.add)

            # sum_d exp(10*sc): ones-matmul then fold the 32 d-tiles
            NV = DT * 41  # 1312
            sv_sb = smalls.tile([1, NV], fp32)
            for n0 in range(0, NV, 512):
                n1 = min(n0 + 512, NV)
                sv_ps = miscpsp.tile([1, 512], fp32, tag="misc")
                nc.tensor.matmul(sv_ps[:, :n1 - n0], ones_b, expv[:, n0:n1],
                                 start=True, stop=True)
                nc.vector.tensor_copy(sv_sb[:, n0:n1], sv_ps[:, :n1 - n0])
            svv = sv_sb[:, :].rearrange("p (t j) -> p t j", t=DT)
            nc.vector.tensor_reduce(
                sumexp[:, NB * SGB:NSUM],
                svv[:, :, 0:NCROPS * NB].rearrange("p t j -> p j t"),
                axis=AX, op=OP.add)

            # ---- region part, per batch element ----
            LNSHIFT = 64
            C1 = float(4.0 * (1.0 / STUDENT_TEMP) * np.log2(np.e))
            for bb in range(NB):
                # teacher E = exp(u) -> fp8e5m2, d-step padded to 400.
                # DMA issue order: first 2 teacher chunks, then the student
                # tensors, then the rest -- keeps both E and xtb early so the
                # D stream (PE critical path) starts ASAP.
                E = ep.tile([128, DT, TGBP], f8e5, tag="E")

                def _teacher_chunk(c):
                    trtc = trtp.tile([128, CHUNK_T, TGB], fp16, tag="trtc")
                    nc.sync.dma_start(
                        out=trtc,
                        in_=trt_d.ap()[c * CHUNK_T * 128:(c + 1) * CHUNK_T * 128,
                                       bb * TGB:(bb + 1) * TGB]
                        .rearrange("(t p) n -> p t n", p=128))
                    nc.scalar.activation(
                        E[:, c * CHUNK_T:(c + 1) * CHUNK_T, 0:TGB],
                        trtc, Exp, scale=1.0)

                _teacher_chunk(0)
                _teacher_chunk(1)
                xtb = xtp.tile([128, DT, SGBP], f8e4, tag="xtb")
                nc.sync.dma_start(
                    out=xtb,
                    in_=xt_d.ap()[:, bb * SGBP:(bb + 1) * SGBP]
                    .rearrange("(t p) s -> p t s", p=128))
                sftb = sftp.tile([128, FT, SGBP], bf16, tag="sftb")
                nc.sync.dma_start(
                    out=sftb,
                    in_=sft_d.ap()[:, bb * SGBP:(bb + 1) * SGBP]
                    .rearrange("(t p) s -> p t s", p=128))
                for c in range(2, DT // CHUNK_T):
                    _teacher_chunk(c)
                tftb = tftp.tile([128, FT, TGB], bf16, tag="tftb")
                nc.sync.dma_start(
                    out=tftb,
                    in_=tftn_d.ap()[:, bb * TGB:(bb + 1) * TGB]
                    .rearrange("(t p) n -> p t n", p=128))

                # DVE bit-trick expx chunks made up-front: keeps the DVE's
                # copy of the work ahead of its select backlog so the lse
                # matmuls never stall on it
                expx_pre = {}
                for c in DVE_CHUNKS:
                    expx = expxp.tile([128, CHUNK_X, SGBP], f8e5, tag="expx")
                    nc.vector.tensor_scalar(
                        expx.bitcast(u8)[:, :, 0:SGB],
                        xtb[:, c * CHUNK_X:(c + 1) * CHUNK_X, 0:SGB],
                        C1, 60.0, op0=OP.mult, op1=OP.add)
                    expx_pre[c] = expx

                zrow = work.tile([1, TGB], bf16, tag="zrow")
                invzbc = work.tile([128, TGB], fp32, tag="invzbc")
                dtiles = {}

                # D + sim matmuls per s-tile (fp8 operands; Z folded via the
                # embedded ones columns).  DoubleRow only fits tiles with
                # <=64 output rows (128-weight-element ISA cap) -> t5 only.
                for ti, (c0, ms, chunk) in enumerate(TILES):
                    n0, w = _chunk_cols(chunk)
                    dps = dpsp.tile([128, TGB], fp32, tag="dps")
                    for d in range(DT):
                        nc.tensor.matmul(dps[:ms, 0:w],
                                         xtb[:, d, c0:c0 + ms],
                                         E[:, d, n0:n0 + w],
                                         start=(d == 0),
                                         stop=(d == DT - 1))
                    sps = spsp.tile([128, TGB], fp32, tag="sps")
                    for f in range(FT):
                        nc.tensor.matmul(sps[:ms, 0:w],
                                         sftb[:, f, c0:c0 + ms],
                                         tftb[:, f, n0:n0 + w],
                                         start=(f == 0), stop=(f == FT - 1))
                    dtiles[ti] = (dps, sps, ms, chunk)

                    if ti == 0:
                        nc.vector.tensor_copy(zrow[:, 0:196], dps[0:1, 0:196])
                    if ti == 1:
                        nc.vector.tensor_copy(zrow[:, 196:392],
                                              dps[0:1, 196:392])
                        bc_ps = miscpsp.tile([128, TGB], fp32, tag="misc")
                        nc.tensor.matmul(bc_ps, ones_row, zrow,
                                         start=True, stop=True)
                        nc.vector.reciprocal(invzbc, bc_ps)

                    # drain completed tiles once invzbc is available (ti>=1)
                    if ti >= 1:
                        for tj in sorted(dtiles):
                            dpsj, spsj, msj, chj = dtiles.pop(tj)
                            col = bb * NST + tj
                            n0j, wj = _chunk_cols(chj)
                            Dn = work.tile([128, TGB], fp32, tag="Dn")
                            nc.vector.tensor_tensor(
                                Dn[:msj, 0:wj], dpsj[:msj, 0:wj],
                                invzbc[:msj, n0j:n0j + wj], op=OP.mult)
                            if chj in (0, 2):
                                m0 = work.tile([128, 1], fp32, tag="m0")
                                nc.vector.tensor_reduce(
                                    m0[:msj], spsj[:msj, 0:196],
                                    axis=AX, op=OP.max)
                                junk = work.tile([128, 196], fp32, tag="junk")
                                nc.vector.scalar_tensor_tensor(
                                    junk[:msj], spsj[:msj, 0:196], m0[:msj],
                                    Dn[:msj, 0:196], op0=OP.is_equal,
                                    op1=OP.mult,
                                    accum_out=ds0[:msj, col:col + 1])
                            if chj in (1, 2):
                                so = 196 if chj == 2 else 0
                                m1 = work.tile([128, 1], fp32, tag="m1")
                                nc.vector.tensor_reduce(
                                    m1[:msj], spsj[:msj, so:so + 196],
                                    axis=AX, op=OP.max)
                                junk1 = work.tile([128, 196], fp32,
                                                  tag="junk1")
                                nc.vector.scalar_tensor_tensor(
                                    junk1[:msj], spsj[:msj, so:so + 196],
                                    m1[:msj], Dn[:msj, so:so + 196],
                                    op0=OP.is_equal, op1=OP.mult,
                                    accum_out=ds1[:msj, col:col + 1])

                # student exps (fp8e5 out) + lse DoubleRow ones-matmuls;
                # after the D/sim stream so the PE isn't gated on expx early
                lse0 = lsepsp.tile([1, 512], fp32, tag="lse")
                lse1 = lsepsp.tile([1, 512], fp32, tag="lse")
                for c in range(DT // CHUNK_X):
                    expx = expxp.tile([128, CHUNK_X, SGBP], f8e5, tag="expx")
                    if c in DVE_CHUNKS:
                        nc.vector.tensor_scalar(
                            expx.bitcast(u8)[:, :, 0:SGB],
                            xtb[:, c * CHUNK_X:(c + 1) * CHUNK_X, 0:SGB],
                            C1, 60.0, op0=OP.mult, op1=OP.add)
                    else:
                        nc.scalar.activation(
                            expx[:, :, 0:SGB],
                            xtb[:, c * CHUNK_X:(c + 1) * CHUNK_X, 0:SGB],
                            Exp, scale=1.0 / STUDENT_TEMP)
                    for jj in range(CHUNK_X // 2):
                        p = c * (CHUNK_X // 2) + jj
                        first, last = (p == 0), (p == DT // 2 - 1)
                        nc.tensor.matmul(
                            lse0[:, 0:HALVES[0][1]], ones_82,
                            expx[:, 2 * jj:2 * jj + 2, 0:384],
                            start=first, stop=last, perf_mode=DR)
                        nc.tensor.matmul(
                            lse1[:, 0:HALVES[1][1]], ones_82,
                            expx[:, 2 * jj:2 * jj + 2, 384:682],
                            start=first, stop=last, perf_mode=DR)
                nc.vector.tensor_copy(
                    sumexp[:, bb * SGB:bb * SGB + 384], lse0[:, 0:384])
                nc.vector.tensor_copy(
                    sumexp[:, bb * SGB + 384:(bb + 1) * SGB], lse1[:, 0:298])

                if bb == NB - 2:
                    # early positive-combine over b0..b2 slots: overlaps the
                    # last batch elem's compute (costs one ACT table swap)
                    NA = (NB - 1) * SGB
                    logsA = smalls.tile([1, NA], fp32)
                    nc.scalar.activation(logsA, sumexp[:, 0:NA], Ln,
                                         scale=2.0 ** -LNSHIFT)
                    junkA = smalls.tile([1, NA], fp32)
                    accA = smalls.tile([1, 1], fp32)
                    nc.vector.scalar_tensor_tensor(
                        junkA, logsA, 1.0, wall_sb[:, 0:NA],
                        op0=OP.mult, op1=OP.mult, accum_out=accA)

            # ---- final combine for the remaining slots
            NA = (NB - 1) * SGB
            logs = smalls.tile([1, NSUM - NA], fp32)
            nc.scalar.activation(logs, sumexp[:, NA:NSUM], Ln,
                                 scale=2.0 ** -LNSHIFT)
            junk_p = smalls.tile([1, NSUM - NA], fp32)
            accB = smalls.tile([1, 1], fp32)
            nc.vector.scalar_tensor_tensor(junk_p, logs, 1.0,
                                           wall_sb[:, NA:NSUM],
                                           op0=OP.mult, op1=OP.mult,
                                           accum_out=accB)
            acc = smalls.tile([1, 1], fp32)
            nc.vector.tensor_tensor(acc, accA, accB, op=OP.add)

            # negative part: dsel columns dotted with packed weights + cls
            junkr0 = smalls.tile([128, NCOL], fp32)
            ra0 = smalls.tile([128, 1], fp32)
            nc.vector.scalar_tensor_tensor(junkr0, ds0, 1.0, w0p_sb,
                                           op0=OP.mult, op1=OP.mult,
                                           accum_out=ra0)
            junkr1 = smalls.tile([128, NCOL], fp32)
            ra1 = smalls.tile([128, 1], fp32)
            nc.vector.scalar_tensor_tensor(junkr1, ds1, 1.0, w1p_sb,
                                           op0=OP.mult, op1=OP.mult,
                                           accum_out=ra1)
            ra = smalls.tile([128, 1], fp32)
            nc.vector.tensor_tensor(ra, ra0, ra1, op=OP.add)
            ra2 = smalls.tile([128, 1], fp32)
            nc.vector.tensor_copy(ra2, ra)
            nc.vector.tensor_tensor(ra2[0:8], ra[0:8], clsneg, op=OP.add)

            neg_ps = miscpsp.tile([1, 1], fp32, tag="misc")
            nc.tensor.matmul(neg_ps, ones_f, ra2, start=True, stop=True)
            wall_np, _, _, _ = _make_weights()
            lncomp = float(LNSHIFT * np.log(2.0) * wall_np.sum())
            accc = smalls.tile([1, 1], fp32)
            nc.vector.tensor_scalar(accc, acc, lncomp, None, op0=OP.add)
            res = smalls.tile([1, 1], fp32)
            nc.vector.tensor_tensor(res, accc, neg_ps, op=OP.subtract)
            nc.sync.dma_start(out=out_d.ap(), in_=res)

    nc.compile()
    return nc, "out"


# ---------------------------------------------------------------------------
# host-side sharding / input prep
# ---------------------------------------------------------------------------

def _col_kinds():
    kinds = []
    for kind, _, ln in SEGS:
        kinds += [kind] * ln
    return kinds


def _make_weights():
    c = 1.0 / (18.0 * 32.0)
    kinds = _col_kinds()[:SGB]
    Wl = np.zeros(SGB, F32)
    W0 = np.zeros(SGB, F32)
    W1 = np.zeros(SGB, F32)
    for k, kind in enumerate(kinds):
        if kind == "c0":
            Wl[k] = 1 * 0.5 * c / NG
            W1[k] = 10.0 * 0.5 * c / NG
        elif kind == "c1":
            Wl[k] = 1 * 0.5 * c / NG
            W0[k] = 10.0 * 0.5 * c / NG
        elif kind == "loc":
            Wl[k] = 2 * 0.5 * c / NL
            W0[k] = 10.0 * 0.5 * c / NL
            W1[k] = 10.0 * 0.5 * c / NL
    wv = np.array([(2 if j >= 2 else 1) * 0.5 * c for j in range(NCROPS)], F32)
    wq = np.zeros((2 * NB, NCROPS * NB), F32)
    for i in range(2):
        for bb in range(NB):
            for j in range(NCROPS):
                if j != i:
                    wq[i * NB + bb, j * NB + bb] = 10.0 * 0.5 * c

    def col_pack(W):  # [682] -> [128, 6] zero-padded, tiled x NB
        w6 = np.zeros((128, NST), F32)
        for ti, (c0, ms, _) in enumerate(TILES):
            w6[:ms, ti] = W[c0:c0 + ms]
        return np.ascontiguousarray(np.tile(w6, (1, NB)))

    wall = np.concatenate([np.tile(Wl, NB), np.repeat(wv, NB)])[None, :]
    return np.ascontiguousarray(wall), col_pack(W0), col_pack(W1), \
        np.ascontiguousarray(wq)


def _dtile_pack(a):
    """[4096, m] -> [128, 32*m] with block t = rows [128t, 128t+128)."""
    d, m = a.shape
    t = d // 128
    return np.ascontiguousarray(
        a.reshape(t, 128, m).transpose(1, 0, 2).reshape(128, t * m))


def _student_rows(bb):
    """Global SR row index per packed column (one b); -1 ones, -2 pad."""
    loc = np.concatenate(
        [np.arange(OFFS[j] + bb * NL, OFFS[j] + (bb + 1) * NL)
         for j in range(2, NCROPS)])
    base = {"c0": np.arange(OFFS[0] + bb * NG, OFFS[0] + (bb + 1) * NG),
            "c1": np.arange(OFFS[1] + bb * NG, OFFS[1] + (bb + 1) * NG),
            "loc": loc}
    idx = []
    for kind, s0, ln in SEGS:
        if kind == "ones":
            idx.append(np.full(ln, -1, np.int64))
        elif kind == "pad":
            idx.append(np.full(ln, -2, np.int64))
        else:
            idx.append(base[kind][s0:s0 + ln])
    return np.concatenate(idx)


def _teacher_rows(bb):
    return np.concatenate([np.arange(bb * NG, (bb + 1) * NG),
                           np.arange(B * NG + bb * NG, B * NG + (bb + 1) * NG)])


def _gather_cols(A, idx, ones_fill, dtype):
    """A [rows, dim] -> [dim, len(idx)] transposed gather.
    idx -1 -> ones_fill, -2 -> 0."""
    out = np.empty((A.shape[1], len(idx)), dtype)
    valid = idx >= 0
    out[:, valid] = A[idx[valid]].T
    out[:, idx == -1] = ones_fill
    out[:, idx == -2] = 0.0
    return np.ascontiguousarray(out)


def _prepare_in_maps(student_cls_pred, student_region_pred, student_feats,
                     teacher_cls_pred, teacher_region_pred, teacher_feats,
                     center, center_grid):
    SR = np.asarray(student_region_pred, F32)
    SF = np.asarray(student_feats, F32)
    TR = np.asarray(teacher_region_pred, F32)
    TF = np.asarray(teacher_feats, F32)
    SC = np.asarray(student_cls_pred, F32)
    TC = np.asarray(teacher_cls_pred, F32)
    center = np.asarray(center, F32).reshape(-1)
    cg = np.asarray(center_grid, F32).reshape(-1)

    if np.any(cg != 0):
        TR = TR - cg[None, :]
    TC = TC - center[None, :]

    # shift-invariance of CE: pre-subtract per-row maxes (cancels exactly)
    SRs = SR - SR.max(1, keepdims=True)
    temp = None  # teacher shift+scale applied per-core below (st known there)

    tfn = TF / np.maximum(np.sqrt((TF * TF).sum(1, keepdims=True)), 1e-12)

    wall, w0p, w1p, wq = _make_weights()

    srows = [_student_rows(bb) for bb in range(B)]
    trows = [_teacher_rows(bb) for bb in range(B)]

    in_maps = []
    for core in range(N_CORES):
        bbs = range(core * NB, (core + 1) * NB)
        sr_idx = np.concatenate([srows[bb] for bb in bbs])
        tr_idx = np.concatenate([trows[bb] for bb in bbs])
        xt = _gather_cols(SRs, sr_idx, 1.0, E4M3)
        sft = _gather_cols(SF, sr_idx, 0.0, BF16)
        TRc = TR[tr_idx]
        u = (TRc - TRc.max(1, keepdims=True)) * _ST_GLOBAL[0]
        trt = np.ascontiguousarray(u.T).astype(F16)
        tftn = np.ascontiguousarray(tfn[tr_idx].T).astype(BF16)
        sc_rows = SC[[j * B + bb for j in range(NCROPS) for bb in bbs]]
        tc_rows = TC[[i * B + bb for i in range(2) for bb in bbs]]
        sc_aug = np.concatenate(
            [sc_rows.T, np.ones((OUT_DIM, 1), F32)], axis=1)  # [4096, 41]
        in_maps.append({
            "xt": xt,
            "trt": trt,
            "sft": sft,
            "tftn": tftn,
            "sctt": _dtile_pack(sc_aug).astype(BF16),
            "tctt": _dtile_pack(np.ascontiguousarray(tc_rows.T)).astype(BF16),
            "wall": wall,
            "w0p": w0p,
            "w1p": w1p,
            "wq": wq,
        })
    return in_maps


_ST_GLOBAL = [1.0 / TEACHER_TEMP]


def _get_program(st):
    key = round(st, 9)
    if key not in _PROG_CACHE:
        _PROG_CACHE[key] = _build_program(st)
    return _PROG_CACHE[key]


def run_cores(inputs, trace=False, **kw):
    """Build+run on 8 cores; returns (partials[8], BassKernelResults)."""
    temp = _temp_from_epoch(inputs["epoch"])
    _ST_GLOBAL[0] = 1.0 / temp
    nc, out_name = _get_program(1.0 / temp)
    in_maps = _prepare_in_maps(
        inputs["student_cls_pred"], inputs["student_region_pred"],
        inputs["student_feats"], inputs["teacher_cls_pred"],
        inputs["teacher_region_pred"], inputs["teacher_feats"],
        inputs["center"], inputs["center_grid"])
    res = run_bass_kernel_spmd(nc, in_maps, core_ids=list(range(N_CORES)),
                               trace=trace, **kw)
    partials = [float(r[out_name].reshape(-1)[0]) for r in res.results]
    return partials, res


def kernel(**inputs) -> np.ndarray:
    assert int(inputs["n_global"]) == NG and int(inputs["n_local"]) == NL
    partials, _ = run_cores(inputs)
    return np.float32(sum(partials))


# revision 26
# speedup vs baseline: 1.2027x; 1.1359x over previous
"""Trainium2 Bass kernel for nn_DDINOLoss (DINO-style distillation loss).

Strategy
--------
Data-parallel over the batch dim (32 batch elems -> 4 per core on 8 cores).
Each core computes a partial scalar loss over its 4 batch elements; the host
sums the 8 partials.

Math (per (i, j) crop pair, teacher chunk i, student crop j != i):
  sum_d -t_d * log_softmax(x)_d = lse(x) - t . x      (since sum_d t_d == 1)
so the cls term needs only lse(v) and q.v, and the region term needs
lse(x_row) and t_sel . x_row where t_sel is the teacher softmax row picked by
the feature-similarity argmax.  The argmax gather is replaced by a
mask-select:  D[s, n] = x_s . E_n  (E = unnormalized teacher exp rows),
dsel[s] = sum_n (sim[s, n] == max_n sim[s, :]) * D[s, n] / Z[n].
Everything becomes matmuls + elementwise ops - no data-dependent memory ops.

v3: fp8 DoubleRow pipeline (all numerics validated on host, ~6e-5 rel err):
 * cross-entropy is shift-invariant in x, so both student and teacher logits
   are sent PRE-SHIFTED by their per-row max (host); the shifts cancel in
   lse - t.x and in D/Z exactly -> no correction constants.
 * shifted x fits fp8e4m3, exp(shifted) fits fp8e5m2: the big D matmul and
   the lse ones-matmuls run in fp8 DoubleRow (2 d-tiles contracted/pass).
 * teacher E = exp(u) emitted by ScalarE directly as fp8e5m2.
 * part of the student exps move off ScalarE to VectorE via a uint8 bit
   trick: u8 = round(40*log2e*x') + 60 bitcast as fp8e5m2 IS 2^(t) with a
   2-bit-mantissa PWL - validated, final rel err 2.4e-5.
 * crop-aware D (crop0 rows need teacher chunk 1 only, crop1 chunk 0 only),
   Z folded into D via embedded all-ones x columns.
 * mask-select fused: scalar_tensor_tensor((sim == max) * Dn, accum_out).
 * single-shot final combine (one Ln) to avoid Exp/Ln ACT-table thrash.
"""

import sys

import numpy as np

if "/opt/trn_rl_repo" not in sys.path:
    sys.path.insert(0, "/opt/trn_rl_repo")

import ml_dtypes

import concourse.bass as bass
import concourse.tile as tile
from concourse import bacc, mybir
from concourse.bass_utils import run_bass_kernel_spmd

BF16 = ml_dtypes.bfloat16
F16 = np.float16
E4M3 = ml_dtypes.float8_e4m3
F32 = np.float32

# ---- problem constants (hardcoded per spec) ----
OUT_DIM = 4096
NCROPS = 10
STUDENT_TEMP = 0.1
WARMUP_TEACHER_TEMP = 0.04
TEACHER_TEMP = 0.07
WARMUP_EPOCHS = 30
NEPOCHS = 100
B = 32
NG = 196
NL = 36
DFEAT = 384
N_CORES = 8
NB = B // N_CORES              # batch elems per core = 4
SPLIT = [NG, NG] + [NL] * (NCROPS - 2)
OFFS = np.cumsum([0] + [s * B for s in SPLIT])
TGB = 2 * NG                       # teacher region rows per batch elem = 392
TGBP = 400                         # E tile padded (16B-aligned d-step)
TG = NB * TGB                      # per-core teacher rows = 1568
DT = OUT_DIM // 128                # 32 d-tiles
FT = DFEAT // 128                  # 3 feature tiles
CHUNK_T = 4                        # d-tiles per teacher exp chunk
CHUNK_X = 8                        # d-tiles per student exp chunk
# which student-exp chunks go to the DVE bit-trick instead of ScalarE
DVE_CHUNKS = (2, 3)

# per-batch-elem student column layout (682 = 680 rows + 2 ones cols),
# padded to 688 for 16B-aligned fp8 d-steps.
SEGS = [("ones", 0, 1), ("c1", 0, 127), ("ones", 0, 1), ("c1", 127, 69),
        ("loc", 0, 58), ("loc", 58, 128), ("loc", 186, 102), ("c0", 0, 26),
        ("c0", 26, 128), ("c0", 154, 42), ("pad", 0, 6)]
SGB = 682                          # real student cols per batch elem
SGBP = 688                         # padded stride
SG = NB * SGBP                     # per-core student cols (padded)
# s-tiles: (col0, ms, chunk)  chunk: 0 -> teacher cols 0:196, 1 -> 196:392,
# 2 -> both.  Z0 lives at (tile0, row 0), Z1 at (tile1, row 0).
TILES = [(0, 128, 0), (128, 128, 2), (256, 128, 2), (384, 128, 2),
         (512, 128, 1), (640, 42, 1)]
NST = len(TILES)
NCOL = NB * NST                    # 24 dsel columns per core
NSUM = NB * SGB + NCROPS * NB      # 2768 log-sum-exp slots
HALVES = [(0, 384), (384, 298)]    # lse column halves

_PROG_CACHE = {}


def _temp_from_epoch(epoch):
    sched = np.concatenate(
        (np.linspace(WARMUP_TEACHER_TEMP, TEACHER_TEMP, WARMUP_EPOCHS),
         np.ones(NEPOCHS - WARMUP_EPOCHS) * TEACHER_TEMP))
    return float(sched[int(epoch)])


def _chunk_cols(chunk):
    """(n0, w) teacher-col range streamed for a tile chunk code."""
    return (0, 196) if chunk == 0 else ((196, 196) if chunk == 1 else (0, 392))


# ---------------------------------------------------------------------------
# device program
# ---------------------------------------------------------------------------

def _build_program(st):
    """st = 1/teacher_temp. Returns (nc, out_name)."""
    fp32 = mybir.dt.float32
    fp16 = mybir.dt.float16
    bf16 = mybir.dt.bfloat16
    f8e4 = mybir.dt.float8e4
    f8e5 = mybir.dt.float8e5
    u8 = mybir.dt.uint8
    Exp = mybir.ActivationFunctionType.Exp
    Ln = mybir.ActivationFunctionType.Ln
    AX = mybir.AxisListType.X
    OP = mybir.AluOpType
    DR = mybir.MatmulPerfMode.DoubleRow

    nc = bacc.Bacc("TRN2", debug=False)

    xt_d = nc.dram_tensor("xt", [OUT_DIM, SG], f8e4, kind="ExternalInput")
    trt_d = nc.dram_tensor("trt", [OUT_DIM, TG], fp16, kind="ExternalInput")
    sft_d = nc.dram_tensor("sft", [DFEAT, SG], bf16, kind="ExternalInput")
    tftn_d = nc.dram_tensor("tftn", [DFEAT, TG], bf16, kind="ExternalInput")
    sct_d = nc.dram_tensor("sctt", [128, DT * 41], bf16, kind="ExternalInput")
    tct_d = nc.dram_tensor("tctt", [128, DT * 8], bf16, kind="ExternalInput")
    wall_d = nc.dram_tensor("wall", [1, NSUM], fp32, kind="ExternalInput")
    w0p_d = nc.dram_tensor("w0p", [128, NCOL], fp32, kind="ExternalInput")
    w1p_d = nc.dram_tensor("w1p", [128, NCOL], fp32, kind="ExternalInput")
    wq_d = nc.dram_tensor("wq", [8, NCROPS * NB], fp32, kind="ExternalInput")
    out_d = nc.dram_tensor("out", [1, 1], fp32, kind="ExternalOutput")

    with tile.TileContext(nc) as tc:
        with (
            tc.tile_pool(name="xtp", bufs=3) as xtp,
            tc.tile_pool(name="trtp", bufs=3) as trtp,
            tc.tile_pool(name="ep", bufs=2) as ep,
            tc.tile_pool(name="expxp", bufs=4) as expxp,
            tc.tile_pool(name="sftp", bufs=3) as sftp,
            tc.tile_pool(name="tftp", bufs=3) as tftp,
            tc.tile_pool(name="smalls", bufs=1) as smalls,
            tc.tile_pool(name="work", bufs=2) as work,
            tc.tile_pool(name="dps", bufs=3, space="PSUM") as dpsp,
            tc.tile_pool(name="sps", bufs=2, space="PSUM") as spsp,
            tc.tile_pool(name="lseps", bufs=2, space="PSUM") as lsepsp,
            tc.tile_pool(name="miscps", bufs=1, space="PSUM") as miscpsp,
        ):
            # ---- constants / small inputs ----
            ones_b = smalls.tile([128, 1], bf16)
            nc.vector.memset(ones_b, 1.0)
            ones_f = smalls.tile([128, 1], fp32)
            nc.vector.memset(ones_f, 1.0)
            ones_row = smalls.tile([1, 128], bf16)
            nc.vector.memset(ones_row, 1.0)
            # lse DoubleRow weights: Ko step must be 16B-aligned, so allocate
            # [128, 2, 16] and slice the first column.
            ones_82f = smalls.tile([128, 2, 16], f8e5)
            nc.vector.memset(ones_82f, 1.0)
            ones_82 = ones_82f[:, :, 0:1]

            w0p_sb = smalls.tile([128, NCOL], fp32)
            nc.sync.dma_start(out=w0p_sb, in_=w0p_d.ap())
            w1p_sb = smalls.tile([128, NCOL], fp32)
            nc.sync.dma_start(out=w1p_sb, in_=w1p_d.ap())
            wq_sb = smalls.tile([8, NCROPS * NB], fp32)
            nc.sync.dma_start(out=wq_sb, in_=wq_d.ap())
            wall_sb = smalls.tile([1, NSUM], fp32)
            nc.sync.dma_start(out=wall_sb, in_=wall_d.ap())

            ds0 = smalls.tile([128, NCOL], fp32)
            nc.vector.memset(ds0, 0.0)
            ds1 = smalls.tile([128, NCOL], fp32)
            nc.vector.memset(ds1, 0.0)
            # sums-of-exps collected here; Ln + weight-reduce at the end
            sumexp = smalls.tile([1, NSUM], fp32)

            # ---- cls part (emitted inside b0, after its DMAs are queued,
            # so the cls matmuls fill the PE while b0's xtb streams in) ----
            clsneg = smalls.tile([8, 1], fp32)

            def _cls_section():
                sct_sb = smalls.tile([128, DT * 41], bf16)
                nc.sync.dma_start(out=sct_sb, in_=sct_d.ap())
                tct_sb = smalls.tile([128, DT * 8], bf16)
                nc.sync.dma_start(out=tct_sb, in_=tct_d.ap())

                qun = smalls.tile([128, DT * 8], bf16)
                nc.scalar.activation(qun, tct_sb, Exp, scale=st)
                expv = smalls.tile([128, DT * 41], bf16)
                nc.scalar.activation(expv, sct_sb, Exp,
                                     scale=1.0 / STUDENT_TEMP)

                # dotq[i, :40] = q_un_i . sc_j ; col 40 = Zq_i (ones in sctt)
                dotq_ps = miscpsp.tile([8, 41], fp32, tag="misc")
                for t in range(DT):
                    nc.tensor.matmul(dotq_ps, qun[:, t * 8:(t + 1) * 8],
                                     sct_sb[:, t * 41:(t + 1) * 41],
                                     start=(t == 0), stop=(t == DT - 1))
                invzq = smalls.tile([8, 1], fp32)
                nc.vector.reciprocal(invzq, dotq_ps[:, 40:41])
                dotn = smalls.tile([8, NCROPS * NB], fp32)
                nc.vector.tensor_scalar(dotn, dotq_ps[:, 0:NCROPS * NB],
                                        invzq, None, op0=OP.mult)
                junkq = smalls.tile([8, NCROPS * NB], fp32)
                nc.vector.tensor_tensor(junkq, dotn, wq_sb, op=OP.mult)
                nc.vector.tensor_reduce(clsneg, junkq, axis=AX, op=OP.add)

                # sum_d exp(10*sc): ones-matmul then fold the 32 d-tiles
                NV = DT * 41  # 1312
                sv_sb = smalls.tile([1, NV], fp32)
                for n0 in range(0, NV, 512):
                    n1 = min(n0 + 512, NV)
                    sv_ps = miscpsp.tile([1, 512], fp32, tag="misc")
                    nc.tensor.matmul(sv_ps[:, :n1 - n0], ones_b,
                                     expv[:, n0:n1], start=True, stop=True)
                    nc.vector.tensor_copy(sv_sb[:, n0:n1], sv_ps[:, :n1 - n0])
                svv = sv_sb[:, :].rearrange("p (t j) -> p t j", t=DT)
                nc.vector.tensor_reduce(
                    sumexp[:, NB * SGB:NSUM],
                    svv[:, :, 0:NCROPS * NB].rearrange("p t j -> p j t"),
                    axis=AX, op=OP.add)

            # ---- region part, per batch element ----
            LNSHIFT = 64
            C1 = float(4.0 * (1.0 / STUDENT_TEMP) * np.log2(np.e))
            for bb in range(NB):
                # teacher E = exp(u) -> fp8e5m2, d-step padded to 400.
                # DMA issue order: first 2 teacher chunks, then the student
                # tensors, then the rest -- keeps both E and xtb early so the
                # D stream (PE critical path) starts ASAP.
                E = ep.tile([128, DT, TGBP], f8e5, tag="E")

                def _teacher_chunk(c):
                    trtc = trtp.tile([128, CHUNK_T, TGB], fp16, tag="trtc")
                    nc.sync.dma_start(
                        out=trtc,
                        in_=trt_d.ap()[c * CHUNK_T * 128:(c + 1) * CHUNK_T * 128,
                                       bb * TGB:(bb + 1) * TGB]
                        .rearrange("(t p) n -> p t n", p=128))
                    nc.scalar.activation(
                        E[:, c * CHUNK_T:(c + 1) * CHUNK_T, 0:TGB],
                        trtc, Exp, scale=1.0)

                _teacher_chunk(0)
                _teacher_chunk(1)
                xtb = xtp.tile([128, DT, SGBP], f8e4, tag="xtb")
                nc.sync.dma_start(
                    out=xtb,
                    in_=xt_d.ap()[:, bb * SGBP:(bb + 1) * SGBP]
                    .rearrange("(t p) s -> p t s", p=128))
                sftb = sftp.tile([128, FT, SGBP], bf16, tag="sftb")
                nc.sync.dma_start(
                    out=sftb,
                    in_=sft_d.ap()[:, bb * SGBP:(bb + 1) * SGBP]
                    .rearrange("(t p) s -> p t s", p=128))
                if bb == 0:
                    _cls_section()
                for c in range(2, DT // CHUNK_T):
                    _teacher_chunk(c)
                tftb = tftp.tile([128, FT, TGB], bf16, tag="tftb")
                nc.sync.dma_start(
                    out=tftb,
                    in_=tftn_d.ap()[:, bb * TGB:(bb + 1) * TGB]
                    .rearrange("(t p) n -> p t n", p=128))

                # DVE bit-trick expx chunks made up-front: keeps the DVE's
                # copy of the work ahead of its select backlog so the lse
                # matmuls never stall on it
                expx_pre = {}
                for c in DVE_CHUNKS:
                    expx = expxp.tile([128, CHUNK_X, SGBP], f8e5, tag="expx")
                    nc.vector.tensor_scalar(
                        expx.bitcast(u8)[:, :, 0:SGB],
                        xtb[:, c * CHUNK_X:(c + 1) * CHUNK_X, 0:SGB],
                        C1, 60.0, op0=OP.mult, op1=OP.add)
                    expx_pre[c] = expx

                zrow = work.tile([1, TGB], bf16, tag="zrow")
                invzbc = work.tile([128, TGB], fp32, tag="invzbc")
                dtiles = {}

                # D + sim matmuls per s-tile (fp8 operands; Z folded via the
                # embedded ones columns).  DoubleRow only fits tiles with
                # <=64 output rows (128-weight-element ISA cap) -> t5 only.
                for ti, (c0, ms, chunk) in enumerate(TILES):
                    n0, w = _chunk_cols(chunk)
                    dps = dpsp.tile([128, TGB], fp32, tag="dps")
                    for d in range(DT):
                        nc.tensor.matmul(dps[:ms, 0:w],
                                         xtb[:, d, c0:c0 + ms],
                                         E[:, d, n0:n0 + w],
                                         start=(d == 0),
                                         stop=(d == DT - 1))
                    sps = spsp.tile([128, TGB], fp32, tag="sps")
                    for f in range(FT):
                        nc.tensor.matmul(sps[:ms, 0:w],
                                         sftb[:, f, c0:c0 + ms],
                                         tftb[:, f, n0:n0 + w],
                                         start=(f == 0), stop=(f == FT - 1))
                    dtiles[ti] = (dps, sps, ms, chunk)

                    if ti == 0:
                        nc.vector.tensor_copy(zrow[:, 0:196], dps[0:1, 0:196])
                    if ti == 1:
                        nc.vector.tensor_copy(zrow[:, 196:392],
                                              dps[0:1, 196:392])
                        bc_ps = miscpsp.tile([128, TGB], fp32, tag="misc")
                        nc.tensor.matmul(bc_ps, ones_row, zrow,
                                         start=True, stop=True)
                        nc.vector.reciprocal(invzbc, bc_ps)

                    # drain completed tiles once invzbc is available (ti>=1)
                    if ti >= 1:
                        for tj in sorted(dtiles):
                            dpsj, spsj, msj, chj = dtiles.pop(tj)
                            col = bb * NST + tj
                            n0j, wj = _chunk_cols(chj)
                            Dn = work.tile([128, TGB], fp32, tag="Dn")
                            nc.vector.tensor_tensor(
                                Dn[:msj, 0:wj], dpsj[:msj, 0:wj],
                                invzbc[:msj, n0j:n0j + wj], op=OP.mult)
                            if chj in (0, 2):
                                m0 = work.tile([128, 1], fp32, tag="m0")
                                nc.vector.tensor_reduce(
                                    m0[:msj], spsj[:msj, 0:196],
                                    axis=AX, op=OP.max)
                                junk = work.tile([128, 196], fp32, tag="junk")
                                nc.vector.scalar_tensor_tensor(
                                    junk[:msj], spsj[:msj, 0:196], m0[:msj],
                                    Dn[:msj, 0:196], op0=OP.is_equal,
                                    op1=OP.mult,
                                    accum_out=ds0[:msj, col:col + 1])
                            if chj in (1, 2):
                                so = 196 if chj == 2 else 0
                                m1 = work.tile([128, 1], fp32, tag="m1")
                                nc.vector.tensor_reduce(
                                    m1[:msj], spsj[:msj, so:so + 196],
                                    axis=AX, op=OP.max)
                                junk1 = work.tile([128, 196], fp32,
                                                  tag="junk1")
                                nc.vector.scalar_tensor_tensor(
                                    junk1[:msj], spsj[:msj, so:so + 196],
                                    m1[:msj], Dn[:msj, so:so + 196],
                                    op0=OP.is_equal, op1=OP.mult,
                                    accum_out=ds1[:msj, col:col + 1])

                # student exps (fp8e5 out) + lse DoubleRow ones-matmuls;
                # after the D/sim stream so the PE isn't gated on expx early
                lse0 = lsepsp.tile([1, 512], fp32, tag="lse")
                lse1 = lsepsp.tile([1, 512], fp32, tag="lse")
                for c in range(DT // CHUNK_X):
                    expx = expxp.tile([128, CHUNK_X, SGBP], f8e5, tag="expx")
                    if c in DVE_CHUNKS:
                        nc.vector.tensor_scalar(
                            expx.bitcast(u8)[:, :, 0:SGB],
                            xtb[:, c * CHUNK_X:(c + 1) * CHUNK_X, 0:SGB],
                            C1, 60.0, op0=OP.mult, op1=OP.add)
                    else:
                        nc.scalar.activation(
                            expx[:, :, 0:SGB],
                            xtb[:, c * CHUNK_X:(c + 1) * CHUNK_X, 0:SGB],
                            Exp, scale=1.0 / STUDENT_TEMP)
                    for jj in range(CHUNK_X // 2):
                        p = c * (CHUNK_X // 2) + jj
                        first, last = (p == 0), (p == DT // 2 - 1)
                        nc.tensor.matmul(
                            lse0[:, 0:HALVES[0][1]], ones_82,
                            expx[:, 2 * jj:2 * jj + 2, 0:384],
                            start=first, stop=last, perf_mode=DR)
                        nc.tensor.matmul(
                            lse1[:, 0:HALVES[1][1]], ones_82,
                            expx[:, 2 * jj:2 * jj + 2, 384:682],
                            start=first, stop=last, perf_mode=DR)
                nc.vector.tensor_copy(
                    sumexp[:, bb * SGB:bb * SGB + 384], lse0[:, 0:384])
                nc.vector.tensor_copy(
                    sumexp[:, bb * SGB + 384:(bb + 1) * SGB], lse1[:, 0:298])

                if bb == NB - 2:
                    # early positive-combine over b0..b2 slots: overlaps the
                    # last batch elem's compute (costs one ACT table swap)
                    NA = (NB - 1) * SGB
                    logsA = smalls.tile([1, NA], fp32)
                    nc.scalar.activation(logsA, sumexp[:, 0:NA], Ln,
                                         scale=2.0 ** -LNSHIFT)
                    junkA = smalls.tile([1, NA], fp32)
                    accA = smalls.tile([1, 1], fp32)
                    nc.vector.scalar_tensor_tensor(
                        junkA, logsA, 1.0, wall_sb[:, 0:NA],
                        op0=OP.mult, op1=OP.mult, accum_out=accA)

            # ---- final combine for the remaining slots
            NA = (NB - 1) * SGB
            logs = smalls.tile([1, NSUM - NA], fp32)
            nc.scalar.activation(logs, sumexp[:, NA:NSUM], Ln,
                                 scale=2.0 ** -LNSHIFT)
            junk_p = smalls.tile([1, NSUM - NA], fp32)
            accB = smalls.tile([1, 1], fp32)
            nc.vector.scalar_tensor_tensor(junk_p, logs, 1.0,
                                           wall_sb[:, NA:NSUM],
                                           op0=OP.mult, op1=OP.mult,
                                           accum_out=accB)
            acc = smalls.tile([1, 1], fp32)
            nc.vector.tensor_tensor(acc, accA, accB, op=OP.add)

            # negative part: dsel columns dotted with packed weights + cls
            junkr0 = smalls.tile([128, NCOL], fp32)
            ra0 = smalls.tile([128, 1], fp32)
            nc.vector.scalar_tensor_tensor(junkr0, ds0, 1.0, w0p_sb,
                                           op0=OP.mult, op1=OP.mult,
                                           accum_out=ra0)
            junkr1 = smalls.tile([128, NCOL], fp32)
            ra1 = smalls.tile([128, 1], fp32)
            nc.vector.scalar_tensor_tensor(junkr1, ds1, 1.0, w1p_sb,
                                           op0=OP.mult, op1=OP.mult,
                                           accum_out=ra1)
            ra = smalls.tile([128, 1], fp32)
            nc.vector.tensor_tensor(ra, ra0, ra1, op=OP.add)
            ra2 = smalls.tile([128, 1], fp32)
            nc.vector.tensor_copy(ra2, ra)
            nc.vector.tensor_tensor(ra2[0:8], ra[0:8], clsneg, op=OP.add)

            neg_ps = miscpsp.tile([1, 1], fp32, tag="misc")
            nc.tensor.matmul(neg_ps, ones_f, ra2, start=True, stop=True)
            wall_np, _, _, _ = _make_weights()
            lncomp = float(LNSHIFT * np.log(2.0) * wall_np.sum())
            accc = smalls.tile([1, 1], fp32)
            nc.vector.tensor_scalar(accc, acc, lncomp, None, op0=OP.add)
            res = smalls.tile([1, 1], fp32)
            nc.vector.tensor_tensor(res, accc, neg_ps, op=OP.subtract)
            nc.sync.dma_start(out=out_d.ap(), in_=res)

    nc.compile()
    return nc, "out"


# ---------------------------------------------------------------------------
# host-side sharding / input prep
# ---------------------------------------------------------------------------

def _col_kinds():
    kinds = []
    for kind, _, ln in SEGS:
        kinds += [kind] * ln
    return kinds


def _make_weights():
    c = 1.0 / (18.0 * 32.0)
    kinds = _col_kinds()[:SGB]
    Wl = np.zeros(SGB, F32)
    W0 = np.zeros(SGB, F32)
    W1 = np.zeros(SGB, F32)
    for k, kind in enumerate(kinds):
        if kind == "c0":
            Wl[k] = 1 * 0.5 * c / NG
            W1[k] = 10.0 * 0.5 * c / NG
        elif kind == "c1":
            Wl[k] = 1 * 0.5 * c / NG
            W0[k] = 10.0 * 0.5 * c / NG
        elif kind == "loc":
            Wl[k] = 2 * 0.5 * c / NL
            W0[k] = 10.0 * 0.5 * c / NL
            W1[k] = 10.0 * 0.5 * c / NL
    wv = np.array([(2 if j >= 2 else 1) * 0.5 * c for j in range(NCROPS)], F32)
    wq = np.zeros((2 * NB, NCROPS * NB), F32)
    for i in range(2):
        for bb in range(NB):
            for j in range(NCROPS):
                if j != i:
                    wq[i * NB + bb, j * NB + bb] = 10.0 * 0.5 * c

    def col_pack(W):  # [682] -> [128, 6] zero-padded, tiled x NB
        w6 = np.zeros((128, NST), F32)
        for ti, (c0, ms, _) in enumerate(TILES):
            w6[:ms, ti] = W[c0:c0 + ms]
        return np.ascontiguousarray(np.tile(w6, (1, NB)))

    wall = np.concatenate([np.tile(Wl, NB), np.repeat(wv, NB)])[None, :]
    return np.ascontiguousarray(wall), col_pack(W0), col_pack(W1), \
        np.ascontiguousarray(wq)


def _dtile_pack(a):
    """[4096, m] -> [128, 32*m] with block t = rows [128t, 128t+128)."""
    d, m = a.shape
    t = d // 128
    return np.ascontiguousarray(
        a.reshape(t, 128, m).transpose(1, 0, 2).reshape(128, t * m))


def _student_rows(bb):
    """Global SR row index per packed column (one b); -1 ones, -2 pad."""
    loc = np.concatenate(
        [np.arange(OFFS[j] + bb * NL, OFFS[j] + (bb + 1) * NL)
         for j in range(2, NCROPS)])
    base = {"c0": np.arange(OFFS[0] + bb * NG, OFFS[0] + (bb + 1) * NG),
            "c1": np.arange(OFFS[1] + bb * NG, OFFS[1] + (bb + 1) * NG),
            "loc": loc}
    idx = []
    for kind, s0, ln in SEGS:
        if kind == "ones":
            idx.append(np.full(ln, -1, np.int64))
        elif kind == "pad":
            idx.append(np.full(ln, -2, np.int64))
        else:
            idx.append(base[kind][s0:s0 + ln])
    return np.concatenate(idx)


def _teacher_rows(bb):
    return np.concatenate([np.arange(bb * NG, (bb + 1) * NG),
                           np.arange(B * NG + bb * NG, B * NG + (bb + 1) * NG)])


def _gather_cols(A, idx, ones_fill, dtype):
    """A [rows, dim] -> [dim, len(idx)] transposed gather.
    idx -1 -> ones_fill, -2 -> 0."""
    out = np.empty((A.shape[1], len(idx)), dtype)
    valid = idx >= 0
    out[:, valid] = A[idx[valid]].T
    out[:, idx == -1] = ones_fill
    out[:, idx == -2] = 0.0
    return np.ascontiguousarray(out)


def _prepare_in_maps(student_cls_pred, student_region_pred, student_feats,
                     teacher_cls_pred, teacher_region_pred, teacher_feats,
                     center, center_grid):
    SR = np.asarray(student_region_pred, F32)
    SF = np.asarray(student_feats, F32)
    TR = np.asarray(teacher_region_pred, F32)
    TF = np.asarray(teacher_feats, F32)
    SC = np.asarray(student_cls_pred, F32)
    TC = np.asarray(teacher_cls_pred, F32)
    center = np.asarray(center, F32).reshape(-1)
    cg = np.asarray(center_grid, F32).reshape(-1)

    if np.any(cg != 0):
        TR = TR - cg[None, :]
    TC = TC - center[None, :]

    # shift-invariance of CE: pre-subtract per-row maxes (cancels exactly)
    SRs = SR - SR.max(1, keepdims=True)
    temp = None  # teacher shift+scale applied per-core below (st known there)

    tfn = TF / np.maximum(np.sqrt((TF * TF).sum(1, keepdims=True)), 1e-12)

    wall, w0p, w1p, wq = _make_weights()

    srows = [_student_rows(bb) for bb in range(B)]
    trows = [_teacher_rows(bb) for bb in range(B)]

    in_maps = []
    for core in range(N_CORES):
        bbs = range(core * NB, (core + 1) * NB)
        sr_idx = np.concatenate([srows[bb] for bb in bbs])
        tr_idx = np.concatenate([trows[bb] for bb in bbs])
        xt = _gather_cols(SRs, sr_idx, 1.0, E4M3)
        sft = _gather_cols(SF, sr_idx, 0.0, BF16)
        TRc = TR[tr_idx]
        u = (TRc - TRc.max(1, keepdims=True)) * _ST_GLOBAL[0]
        trt = np.ascontiguousarray(u.T).astype(F16)
        tftn = np.ascontiguousarray(tfn[tr_idx].T).astype(BF16)
        sc_rows = SC[[j * B + bb for j in range(NCROPS) for bb in bbs]]
        tc_rows = TC[[i * B + bb for i in range(2) for bb in bbs]]
        sc_aug = np.concatenate(
            [sc_rows.T, np.ones((OUT_DIM, 1), F32)], axis=1)  # [4096, 41]
        in_maps.append({
            "xt": xt,
            "trt": trt,
            "sft": sft,
            "tftn": tftn,
            "sctt": _dtile_pack(sc_aug).astype(BF16),
            "tctt": _dtile_pack(np.ascontiguousarray(tc_rows.T)).astype(BF16),
            "wall": wall,
            "w0p": w0p,
            "w1p": w1p,
            "wq": wq,
        })
    return in_maps


_ST_GLOBAL = [1.0 / TEACHER_TEMP]


def _get_program(st):
    key = round(st, 9)
    if key not in _PROG_CACHE:
        _PROG_CACHE[key] = _build_program(st)
    return _PROG_CACHE[key]


def run_cores(inputs, trace=False, **kw):
    """Build+run on 8 cores; returns (partials[8], BassKernelResults)."""
    temp = _temp_from_epoch(inputs["epoch"])
    _ST_GLOBAL[0] = 1.0 / temp
    nc, out_name = _get_program(1.0 / temp)
    in_maps = _prepare_in_maps(
        inputs["student_cls_pred"], inputs["student_region_pred"],
        inputs["student_feats"], inputs["teacher_cls_pred"],
        inputs["teacher_region_pred"], inputs["teacher_feats"],
        inputs["center"], inputs["center_grid"])
    res = run_bass_kernel_spmd(nc, in_maps, core_ids=list(range(N_CORES)),
                               trace=trace, **kw)
    partials = [float(r[out_name].reshape(-1)[0]) for r in res.results]
    return partials, res


def kernel(**inputs) -> np.ndarray:
    assert int(inputs["n_global"]) == NG and int(inputs["n_local"]) == NL
    partials, _ = run_cores(inputs)
    return np.float32(sum(partials))
